# revision 3
# baseline (speedup 1.0000x reference)
"""Trainium2 Bass kernel for nn_Attention (non-local-block style attention).

Reference computation (per batch b, z flattened to [Ci, N], N = T*H*W = 4096):
    theta = w_theta @ z + b_theta        [Co, N]
    phi   = w_phi   @ z + b_phi          [Co, N]
    psi   = w_psi   @ z + b_psi          [Co, N]
    g[n,m]   = sum_c phi[c,n] psi[c,m]
    G        = relu(g / N)
    tmp[c,n] = sum_m G[n,m] theta[c,m]
    out      = w_v @ tmp + b_v + z       [Ci, N]

Sharding: 8 cores = 2 batches x 4 token-blocks of N/4=1024. Each core gets the
full z[b] (needed for psi/theta over all m), host-ROTATED so its own token
block sits in columns 0:1024 -- the attention reductions sum over all m, so a
permutation of m changes nothing as long as psi and theta^T use the same
order. One shared program for all 8 cores; fully data-parallel SPMD.

Per-core dataflow (all matmuls bf16 into fp32 PSUM; 1/N folded into w_psi and
b_psi host-side; the residual uses the bf16 z copy, well within the 2e-2
tolerance):

  psi_dup [128, 4096]: rows 0-63 == rows 64-127 == psi (host-duplicated weight
          columns give both PE row-groups their operands)
  phi_dup [128, 1024]: same for phi on the own (first) token block
  thT     [128, 32*64]: theta^T tiles (m on partitions), theta bias added via
          ones-row matmul prefill of each PSUM bank
  per m-tile (32):
    gT [128, 1024] f32 PSUM (2 banks) = psi^T phi via a row-group pair
       ((0,0)/(64,0)), both halves concurrent on the full array
    G = relu(gT): ONE whole-tile PSUM->SBUF bf16 op, tiles alternating between
       ScalarE and VectorE (whole-tile ops amortize the fixed per-op cost and
       halve semaphore traffic vs splitting every tile across both engines;
       fp32 PSUM reads are 1x on either engine, so the PSUM read port is the
       floor -- bf16 PSUM matmul output would give DVE 2x but is TRN3-only)
    tmp [128, 512] += thT^T G: col-group pair (0,0)/(0,64) accumulating into
       one PSUM bank (rows 0:64 = n-chunk0, 64:128 = n-chunk1)
  vg = w_v^T tmp (bf16 row-group pair); out = vg + b_v + z_blk; DMA.

Schedule: the PE stream is kept dense from first weight arrival to loop end so
the HAM clock gate (1.2 -> 2.4 GHz after ~3.4us of sustained activity) fires
once and never re-throttles: a short warmup burst on the weight pack bridges
until z chunks land, then DMA-paced projections (per 1024-column z chunk:
psi chunk, theta^T group; phi first from chunk 0), then the attention loop in
group-of-2 phases (tmp,tmp,g,g) with 3 g-tiles of PSUM lookahead so
same-structure matmuls pipeline and the array-footprint switch between the g
(row-split) and tmp (col-split) weight sets happens once per 2 m-tiles. A
1-column dummy Relu right after the bias pack lands pulls the one-time ~2.7us
ACT table load into the DMA phase.
"""

import ml_dtypes
import numpy as np

import concourse.bacc as bacc
import concourse.mybir as mybir
import concourse.tile as tile
from concourse.bass_utils import run_bass_kernel_spmd

F32 = mybir.dt.float32
BF16 = mybir.dt.bfloat16
AF = mybir.ActivationFunctionType
ALU = mybir.AluOpType
BF16NP = ml_dtypes.bfloat16

B, CI, CO = 2, 128, 64
T, H, W = 4, 32, 32
N = T * H * W            # 4096 tokens
NCORES = 8
BLK = N // (NCORES // B)  # 1024 tokens per core
CH = 512                 # psum-bank chunk (fp32)
MT = N // 128            # 32 m-tiles
NWARM = 4                # warmup matmuls (F=320) before the first z chunk lands

_CACHE = {}


def _build():
    nc = bacc.Bacc("TRN2", target_bir_lowering=False, debug=False)

    zb16 = nc.dram_tensor("zb16", [CI, N], BF16, kind="ExternalInput")
    wpack = nc.dram_tensor("wpack", [CI, 320], BF16, kind="ExternalInput")
    smallpack = nc.dram_tensor("smallpack", [1, 640], BF16, kind="ExternalInput")
    biaspack = nc.dram_tensor("biaspack", [CI, 4], F32, kind="ExternalInput")
    wv2 = nc.dram_tensor("wv2", [128, CI], BF16, kind="ExternalInput")
    out = nc.dram_tensor("out", [CI, BLK], F32, kind="ExternalOutput")

    with tile.TileContext(nc) as tc:
        with (
            tc.tile_pool(name="const", bufs=1) as cpool,
            tc.tile_pool(name="zp", bufs=1) as zp,
            tc.tile_pool(name="proj", bufs=1) as pp,
            tc.tile_pool(name="gs", bufs=4) as gp,
            tc.tile_pool(name="tail", bufs=2) as tailp,
            tc.tile_pool(name="pst", bufs=1, space="PSUM") as pst,
        ):
            # ---- input DMAs, smallest first, z chunks in consumption order
            biaspack_sb = cpool.tile([CI, 4], F32)
            nc.sync.dma_start(biaspack_sb[:], biaspack[:])
            smallpack_sb = cpool.tile([1, 640], BF16)
            nc.sync.dma_start(smallpack_sb[:], smallpack[:])
            wpack_sb = cpool.tile([CI, 320], BF16)
            nc.sync.dma_start(wpack_sb[:], wpack[:])
            zb16_sb = zp.tile([CI, N], BF16)
            for j in range(4):
                nc.sync.dma_start(
                    zb16_sb[:, j * 1024:(j + 1) * 1024],
                    zb16[:, j * 1024:(j + 1) * 1024],
                )
            # tail-only weight rides the (slow-starting) gpsimd queue
            wv_sb = cpool.tile([128, CI], BF16)
            nc.gpsimd.dma_start(wv_sb[:], wv2[:])

            wpsiT2_sb = wpack_sb[:, 0:128]
            wphiT2_sb = wpack_sb[:, 128:256]
            wthetaT_sb = wpack_sb[:, 256:320]
            btheta8_sb = smallpack_sb[:, 0:CH]
            ones_sb = smallpack_sb[:, CH:CH + CI]
            bpsi_sb = biaspack_sb[:, 0:1]
            bphi_sb = biaspack_sb[:, 1:2]
            bv_sb = biaspack_sb[:, 2:3]
            zero_sb = biaspack_sb[:, 3:4]

            # early dummy Relu: forces the one-time ACT table load to overlap
            # the DMA phase instead of blocking the first projection evac
            scratch_sb = cpool.tile([CI, 1], BF16)
            nc.scalar.activation(scratch_sb[:], zero_sb, AF.Relu)

            # tmp accumulator: one PSUM bank, col-packed
            # (rows 0:64 = tmp[:, 0:512], rows 64:128 = tmp[:, 512:1024])
            tmp_ps = pst.tile([128, CH], F32)

            psi_sb = pp.tile([128, N], BF16)
            phi_sb = pp.tile([128, BLK], BF16)
            thT_sb = pp.tile([128, MT * CO], BF16)

            # ---- HAM ignition: keep the PE busy from wpack arrival until the
            # first z chunk lands; the DMA-paced projections take over from
            # there and the activity monitor un-throttles ~3.4us in.
            with tc.tile_pool(name="warm", bufs=1, space="PSUM") as wpool:
                wps = wpool.tile([128, 320], F32)
                for _ in range(NWARM):
                    nc.tensor.matmul(
                        wps[:], wpsiT2_sb, wpack_sb[:], skip_group_check=True
                    )

            # ---- projections, paced by the four zb16 chunk DMAs.
            # PSUM tiles recycle through a 2-buf pool; evacuations alternate
            # between ScalarE and VectorE so neither becomes the pacer.
            with tc.tile_pool(name="psj", bufs=2, space="PSUM") as psj:
                def emit_phi(j, eng):
                    ps = psj.tile([128, CH], F32, tag="m", name=f"phi{j}")
                    nc.tensor.matmul(
                        ps[:], wphiT2_sb, zb16_sb[:, j * CH:(j + 1) * CH]
                    )
                    dst = phi_sb[:, j * CH:(j + 1) * CH]
                    if eng == "A":
                        nc.scalar.activation(dst, ps[:], AF.Identity, bias=bphi_sb)
                    else:
                        nc.vector.tensor_scalar_add(dst, ps[:], bphi_sb)

                def emit_psi(j, eng):
                    ps = psj.tile([128, CH], F32, tag="m", name=f"psi{j}")
                    nc.tensor.matmul(
                        ps[:], wpsiT2_sb, zb16_sb[:, j * CH:(j + 1) * CH]
                    )
                    dst = psi_sb[:, j * CH:(j + 1) * CH]
                    if eng == "A":
                        nc.scalar.activation(dst, ps[:], AF.Identity, bias=bpsi_sb)
                    else:
                        nc.vector.tensor_scalar_add(dst, ps[:], bpsi_sb)

                def emit_thT(grp, eng):
                    ps = psj.tile([128, CH], F32, tag="m", name=f"th{grp}")
                    nc.tensor.matmul(
                        ps[:], ones_sb, btheta8_sb,
                        start=True, stop=False, skip_group_check=True,
                    )
                    for j in range(8):
                        mi = grp * 8 + j
                        nc.tensor.matmul(
                            ps[:, j * CO:(j + 1) * CO],
                            zb16_sb[:, mi * 128:(mi + 1) * 128],
                            wthetaT_sb,
                            start=False, stop=(j == 7), skip_group_check=True,
                        )
                    dst = thT_sb[:, grp * CH:(grp + 1) * CH]
                    if eng == "A":
                        nc.scalar.activation(dst, ps[:], AF.Copy)
                    else:
                        nc.vector.tensor_copy(dst, ps[:])

                emit_phi(0, "A")
                emit_phi(1, "V")
                for grp in range(4):
                    emit_psi(2 * grp, "V" if grp % 2 == 0 else "A")
                    emit_psi(2 * grp + 1, "A" if grp % 2 == 0 else "V")
                    emit_thT(grp, "A" if grp % 2 == 0 else "V")

            # ---- attention loop: 32 m-tiles, group-of-2 phases, lookahead 3
            with tc.tile_pool(name="psg", bufs=3, space="PSUM") as psg:
                gsb = {}

                def emit_g(mt):
                    gps = psg.tile([128, 2 * CH], F32, tag="g", name=f"g{mt}")
                    msl = slice(mt * 128, (mt + 1) * 128)
                    nc.tensor.matmul(
                        gps[:, 0:CH],
                        psi_sb[0:CO, msl],
                        phi_sb[0:CO, 0:CH],
                        tile_position=(0, 0),
                    )
                    nc.tensor.matmul(
                        gps[:, CH:2 * CH],
                        psi_sb[CO:128, msl],
                        phi_sb[CO:128, CH:2 * CH],
                        tile_position=(64, 0),
                    )
                    s = gp.tile([128, 2 * CH], BF16, tag="gs", name=f"s{mt}")
                    # whole-tile evac; ScalarE is the slightly faster engine,
                    # so it takes the odd extra tile (17 A / 15 V)
                    if mt % 2 == 0 or mt in (15, 31):
                        nc.scalar.activation(s[:], gps[:], AF.Relu)
                    else:
                        nc.vector.tensor_scalar_max(s[:], gps[:], 0.0)
                    gsb[mt] = s

                def emit_tmp(mt):
                    s = gsb.pop(mt)
                    lhs = thT_sb[:, mt * CO:(mt + 1) * CO]
                    nc.tensor.matmul(
                        tmp_ps[0:CO, :], lhs, s[:, 0:CH],
                        start=(mt == 0), stop=(mt == MT - 1),
                        tile_position=(0, 0), skip_group_check=True,
                    )
                    nc.tensor.matmul(
                        tmp_ps[CO:128, :], lhs, s[:, CH:2 * CH],
                        start=(mt == 0), stop=(mt == MT - 1),
                        tile_position=(0, 64), skip_group_check=True,
                    )

                emit_g(0)
                emit_g(1)
                emit_g(2)
                nxt = 3
                for mt in range(MT):
                    emit_tmp(mt)
                    if mt % 2 == 1:
                        for _ in range(2):
                            if nxt < MT:
                                emit_g(nxt)
                                nxt += 1

            # ---- tail: tmp -> SBUF bf16, vg = w_v^T tmp (row-group pair),
            # out = vg + b_v + z_blk, DMA out per 512-chunk ----
            with tc.tile_pool(name="psv", bufs=2, space="PSUM") as psv:
                tmp_sb = tailp.tile([128, CH], BF16, tag="tmp")
                nc.scalar.activation(tmp_sb[:], tmp_ps[:], AF.Copy)
                vgA = psv.tile([CI, CH], F32, tag="v", name="vgA")
                vgB = psv.tile([CI, CH], F32, tag="v", name="vgB")
                nc.tensor.matmul(
                    vgA[:], wv_sb[0:CO, :], tmp_sb[0:CO, :], tile_position=(0, 0)
                )
                nc.tensor.matmul(
                    vgB[:], wv_sb[CO:128, :], tmp_sb[CO:128, :],
                    tile_position=(64, 0),
                )
                for h, vg_ps in ((0, vgA), (1, vgB)):
                    out_sb = tailp.tile([CI, CH], F32, tag="os", name=f"os{h}")
                    nc.vector.scalar_tensor_tensor(
                        out_sb[:],
                        vg_ps[:],
                        bv_sb,
                        zb16_sb[:, h * CH:(h + 1) * CH],
                        ALU.add,
                        ALU.add,
                    )
                    nc.sync.dma_start(out[:, h * CH:(h + 1) * CH], out_sb[:])

    nc.compile()
    return nc


def _get_nc():
    if "nc" not in _CACHE:
        _CACHE["nc"] = _build()
    return _CACHE["nc"]


def build_in_maps(z, w_theta, b_theta, w_phi, b_phi, w_psi, b_psi, w_v, b_v):
    z = np.asarray(z, dtype=np.float32)
    z2 = np.ascontiguousarray(z.reshape(B, CI, N))
    z216 = z2.astype(BF16NP)

    sc = np.float32(1.0 / N)
    wpsiT = np.asarray(w_psi, np.float32).T * sc
    wphiT = np.asarray(w_phi, np.float32).T
    wthetaT = np.asarray(w_theta, np.float32).T
    wpack = np.ascontiguousarray(
        np.concatenate(
            [wpsiT, wpsiT, wphiT, wphiT, wthetaT], axis=1
        ).astype(BF16NP)
    )
    smallpack = np.zeros((1, 640), dtype=BF16NP)
    smallpack[0, 0:CH] = np.tile(np.asarray(b_theta, np.float32), 8).astype(BF16NP)
    smallpack[0, CH:CH + CI] = np.ones(CI, dtype=BF16NP)
    biaspack = np.stack(
        [
            np.concatenate([b_psi, b_psi]).astype(np.float32) * sc,
            np.concatenate([b_phi, b_phi]).astype(np.float32),
            np.asarray(b_v, np.float32),
            np.zeros(CI, np.float32),
        ],
        axis=1,
    ).astype(np.float32)
    wvT1 = np.asarray(w_v, np.float32).T
    wv2 = np.ascontiguousarray(
        np.concatenate([wvT1, wvT1], axis=0).astype(BF16NP)
    )

    in_maps = []
    for core in range(NCORES):
        b, nb = divmod(core, NCORES // B)
        # rotate so this core's token block sits in columns 0:BLK; the m
        # reductions are permutation-invariant, so psi/theta built from the
        # rotated z are consistent with it
        zrot = np.ascontiguousarray(np.roll(z216[b], -nb * BLK, axis=1))
        in_maps.append(
            {
                "zb16": zrot,
                "wpack": wpack,
                "smallpack": smallpack,
                "biaspack": biaspack,
                "wv2": wv2,
            }
        )
    return in_maps


def kernel(z, w_theta, b_theta, w_phi, b_phi, w_psi, b_psi, w_v, b_v):
    in_maps = build_in_maps(
        z, w_theta, b_theta, w_phi, b_phi, w_psi, b_psi, w_v, b_v
    )
    nc = _get_nc()
    res = run_bass_kernel_spmd(nc, in_maps, core_ids=list(range(NCORES)))

    out_full = np.empty((B, CI, N), dtype=np.float32)
    for core in range(NCORES):
        b, nb = divmod(core, NCORES // B)
        out_full[b][:, nb * BLK:(nb + 1) * BLK] = res.results[core]["out"]
    return out_full.reshape(B, CI, T, H, W)
